# revision 3
# baseline (speedup 1.0000x reference)
"""Trainium2 Bass kernel for strictly-causal RoPE self-attention (no softmax).

  out[b,h] = tril(rope(Q)@rope(Q)^T, -1) @ V    with K = Q.

Sharding: B*H = 8 independent (b,h) slices -> one per NeuronCore (pure data
parallel, no collectives). Per core: T=N=2048.

Per-core algorithm (all device compute, bf16 matmul / f32 accumulate):
  - Host passes Q pre-transposed+deinterleaved (layout prep only):
      qte[n',t] = Q[t,2n'], qto[n',t] = Q[t,2n'+1]   [N/2, T]
    plus RoPE cos/sin tables in the same layout (input-independent constants;
    freqs are pair-constant so one table serves even+odd lanes).
  - Device RoPE (DVE):  qrt_e = qte*cos - qto*sin ; qrt_o = qto*cos + qte*sin
    giving QRT = rope(Q)^T as 16 [128, T] bf16 tiles (n on partitions).
  - Stage 1 (PE): P[s,t] = sum_n QRT[n,s]*QRT[n,t]  for lower-triangle blocks,
    in column-supersteps of 4 t-blocks; strict-causal mask applied to the
    diagonal 128x128 block during the PSUM->SBUF evict.
  - Stage 2 (PE): out[t,n] += P[s,t]^T @ V[s,n], accumulating j-blocks in PSUM.
"""

import os
import sys
import math

for _p in ("/opt/trn_rl_repo", "/root/.axon_site/_ro/trn_rl_repo"):
    if os.path.isdir(_p) and _p not in sys.path:
        sys.path.append(_p)

import numpy as np
import ml_dtypes

B, H, T, N = 2, 4, 2048, 2048
THETA = 2.0 ** 16
NCORES = 8

bf16 = ml_dtypes.bfloat16

LAST_RESULT = None  # BassKernelResults of the most recent run (for test.py)


def build_bass(t_len=T, n_dim=N, num_devices=NCORES):
    from concourse import bacc, mybir, tile

    nc = bacc.Bacc("TRN2", target_bir_lowering=False, debug=False,
                   num_devices=num_devices)
    bf = mybir.dt.bfloat16
    f32 = mybir.dt.float32
    mult = mybir.AluOpType.mult

    nh = n_dim // 2
    kh = nh // 128          # n-tiles per half (8)
    kk_n = n_dim // 128     # total n-tiles (16)
    nb = t_len // 128       # t-blocks (16)
    sw = min(4, nb)         # superstep width in t-blocks
    cw = min(512, n_dim)    # output n-chunk width
    nch = n_dim // cw       # output n-chunks

    qte = nc.declare_dram_parameter("qte", [nh, t_len], bf, isOutput=False)
    qto = nc.declare_dram_parameter("qto", [nh, t_len], bf, isOutput=False)
    cosd = nc.declare_dram_parameter("cosT", [nh, t_len], bf, isOutput=False)
    sind = nc.declare_dram_parameter("sinT", [nh, t_len], bf, isOutput=False)
    vin = nc.declare_dram_parameter("v", [t_len, n_dim], bf, isOutput=False)
    maskd = nc.declare_dram_parameter("mask", [128, 128], f32, isOutput=False)
    outd = nc.declare_dram_parameter("out", [t_len, n_dim], f32, isOutput=True)

    with tile.TileContext(nc) as tc:
        with (
            tc.tile_pool(name="qrt", bufs=kk_n) as qrt_pool,
            tc.tile_pool(name="vres", bufs=nb) as v_pool,
            tc.tile_pool(name="tbl", bufs=4) as tbl_pool,
            tc.tile_pool(name="rtmp", bufs=4) as tmp_pool,
            tc.tile_pool(name="ptile", bufs=20) as p_pool,
            tc.tile_pool(name="osb", bufs=4) as out_pool,
            tc.tile_pool(name="mk", bufs=1) as mk_pool,
            tc.tile_pool(name="ps1", bufs=4, space="PSUM") as ps1_pool,
            tc.tile_pool(name="ps2", bufs=4, space="PSUM") as ps2_pool,
        ):
            mask_sb = mk_pool.tile([128, 128], f32)
            nc.sync.dma_start(mask_sb[:], maskd[:])

            v_tiles = []
            for jb in range(nb):
                vt = v_pool.tile([128, n_dim], bf)
                nc.sync.dma_start(vt[:], vin[128 * jb:128 * (jb + 1), :])
                v_tiles.append(vt)

            # RoPE: build QRT tiles (index kk: 0..kh-1 even-half, kh..2kh-1 odd)
            qrt = [None] * kk_n
            for kk in range(kh):
                te = qrt_pool.tile([128, t_len], bf, tag="qrt")
                to = qrt_pool.tile([128, t_len], bf, tag="qrt")
                ct = tbl_pool.tile([128, t_len], bf, tag="tbl")
                st = tbl_pool.tile([128, t_len], bf, tag="tbl")
                nc.sync.dma_start(te[:], qte[128 * kk:128 * (kk + 1), :])
                nc.sync.dma_start(to[:], qto[128 * kk:128 * (kk + 1), :])
                nc.sync.dma_start(ct[:], cosd[128 * kk:128 * (kk + 1), :])
                nc.sync.dma_start(st[:], sind[128 * kk:128 * (kk + 1), :])
                t_os = tmp_pool.tile([128, t_len], bf, tag="tmp")
                t_es = tmp_pool.tile([128, t_len], bf, tag="tmp")
                nc.vector.tensor_mul(t_os[:], to[:], st[:])   # O*S
                nc.vector.tensor_mul(t_es[:], te[:], st[:])   # E*S
                nc.vector.tensor_mul(te[:], te[:], ct[:])     # E*C (in place)
                nc.vector.tensor_sub(te[:], te[:], t_os[:])   # -> QRT_E
                nc.vector.tensor_mul(to[:], to[:], ct[:])     # O*C (in place)
                nc.vector.tensor_add(to[:], to[:], t_es[:])   # -> QRT_O
                qrt[kk] = te
                qrt[kh + kk] = to

            # main loop: supersteps of `sw` t-blocks
            for ic in range(nb // sw):
                t0 = sw * 128 * ic
                t1 = sw * 128 * (ic + 1)
                ptiles = {}
                for j in range(sw * ic + sw):
                    rj0 = max(128 * j, t0)
                    w = t1 - rj0
                    ps = ps1_pool.tile([128, w], f32)
                    for kk in range(kk_n):
                        nc.tensor.matmul(
                            ps[:, :],
                            qrt[kk][:, 128 * j:128 * j + 128],
                            qrt[kk][:, rj0:t1],
                            start=(kk == 0),
                            stop=(kk == kk_n - 1),
                        )
                    pt = p_pool.tile([128, w], bf)
                    if 128 * j >= t0:
                        # diagonal block: strict-causal mask (keep s < t)
                        nc.vector.tensor_tensor(pt[:, 0:128], ps[:, 0:128],
                                                mask_sb[:], mult)
                        if w > 128:
                            nc.vector.tensor_copy(pt[:, 128:w], ps[:, 128:w])
                    else:
                        nc.vector.tensor_copy(pt[:, :], ps[:, :])
                    ptiles[j] = (pt, rj0)
                for d in range(sw):
                    i = sw * ic + d
                    ti = 128 * i
                    for ch in range(nch):
                        ops = ps2_pool.tile([128, cw], f32)
                        for j in range(i + 1):
                            pt, rj0 = ptiles[j]
                            off = ti - rj0
                            nc.tensor.matmul(
                                ops[:, :],
                                pt[:, off:off + 128],
                                v_tiles[j][:, cw * ch:cw * (ch + 1)],
                                start=(j == 0),
                                stop=(j == i),
                            )
                        osb = out_pool.tile([128, cw], f32)
                        nc.scalar.copy(osb[:], ops[:])
                        nc.sync.dma_start(
                            outd[ti:ti + 128, cw * ch:cw * (ch + 1)], osb[:])

    nc.compile()
    return nc


def _tables(t_len=T, n_dim=N):
    t = np.arange(n_dim, dtype=np.float32)
    q = np.floor(t / 2.0) * 2.0
    f = (1.0 / THETA ** (q.astype(np.float64) / n_dim)
         / (2.0 * math.pi)).astype(np.float32)
    phases = np.arange(t_len, dtype=np.float32)[:, None] * f[None, :]
    ph = (phases % 1.0) * np.float32(2.0 * math.pi)
    ct = np.ascontiguousarray(np.cos(ph)[:, 0::2].T).astype(bf16)  # [N/2, T]
    st = np.ascontiguousarray(np.sin(ph)[:, 0::2].T).astype(bf16)
    return ct, st


def _mask128():
    s = np.arange(128, dtype=np.float32)[:, None]
    tt = np.arange(128, dtype=np.float32)[None, :]
    return (s < tt).astype(np.float32)


_compiled = {}


def _get_nc():
    if "nc" not in _compiled:
        _compiled["nc"] = build_bass()
    return _compiled["nc"]


def kernel(Q, V):
    global LAST_RESULT
    from concourse.bass_utils import run_bass_kernel_spmd

    Q = np.asarray(Q)
    V = np.asarray(V)
    assert Q.shape == (B, H, T, N) and V.shape == (B, H, T, N)

    nc = _get_nc()
    ct, st = _tables()
    mask = _mask128()

    in_maps = []
    for b in range(B):
        for h in range(H):
            qs = Q[b, h]
            in_maps.append({
                "qte": np.ascontiguousarray(qs[:, 0::2].T).astype(bf16),
                "qto": np.ascontiguousarray(qs[:, 1::2].T).astype(bf16),
                "cosT": ct,
                "sinT": st,
                "v": V[b, h].astype(bf16),
                "mask": mask,
            })

    res = run_bass_kernel_spmd(nc, in_maps, core_ids=list(range(NCORES)))
    LAST_RESULT = res

    out = np.empty((B, H, T, N), dtype=np.float32)
    for b in range(B):
        for h in range(H):
            out[b, h] = res.results[b * H + h]["out"]
    return out
